# revision 10
# baseline (speedup 1.0000x reference)
"""HRA (Householder Reflection Adaptation) forward kernel for Trainium2.

Math: out = x @ Q with Q = prod_i (I - 2 u_i u_i^T), u_i = normalized columns
of hra_u [4096, 8].  Using the compact WY representation:
    Q = I - U T U^T      (T upper-triangular 8x8, diag=2)
    out = x - (x @ A) @ U^T,   A = U @ T
so the device only does two skinny matmuls per tile plus a subtract.

Sharding: data-parallel over rows. x [4,2048,4096] -> [8192, 4096]; each of
8 cores gets 1024 contiguous rows. A and U^T are tiny and replicated.

Everything runs in bf16 (inputs quantized host-side; |err| ~ 5e-3 rel, well
inside the 2e-2 gate): HBM traffic halves (16.8 MB/core, ~47 us roofline)
and PE transposes run at 1 cycle/row (vs 2 for f32).

The limiting resource is the PSUM->SBUF drain (transposed x strips + the
dense rank-8 correction, ~1 elem/lane/cycle on ACT/DVE only).  Three
mitigations:
  - chunks 24-31 of x^T are loaded DIRECTLY from DRAM via the HWDGE xbar
    DMA-transpose (bf16-only path) on the ACT ring: those chunks need no
    PE transpose, no PSUM, and no ACT copy, at the cost of +2.1 MB HBM
    reads that fit in otherwise-idle DMA capacity
  - the correction drain is spread over THREE engines: direct DVE
    subtracts, ACT-copy + GPSIMD bf16 subtracts (GPSIMD is otherwise
    idle), and ACT-copy + cheap 2x bf16 DVE subtracts
  - output DMAs issue from GPSIMD (SWDGE) so descriptor generation costs
    no ACT/DVE time; the last block streams out in quarter pieces
"""

import os
import sys

for _p in ("/opt/trn_rl_repo", "/root/.axon_site", "/root/.axon_site/_ro/trn_rl_repo",
           "/root/.axon_site/_ro/pypackages"):
    if os.path.isdir(_p) and _p not in sys.path:
        sys.path.append(_p)

import numpy as np
import ml_dtypes

import concourse.bass as bass
import concourse.mybir as mybir
import concourse.tile as tile
from concourse import bacc
from concourse.bass_utils import run_bass_kernel_spmd

B, S, D, R = 4, 2048, 4096, 8
N_CORES = 8
ROWS = B * S                      # 8192
ROWS_PER_CORE = ROWS // N_CORES   # 1024
P = 128
D_CHUNKS = 32                     # 128-wide d chunks
SG = 8                            # d-chunks per PE transpose super-group
PE_SGS = 3                        # super-groups 0-2 via PE (chunks 0-23)
DMA_CH0 = PE_SGS * SG             # chunks 24-31 arrive via DMA transpose
UPD_CHUNKS = 8                    # 512-wide update units per j

F32 = mybir.dt.float32
BF16 = mybir.dt.bfloat16
NP_BF16 = ml_dtypes.bfloat16

_CACHE = {}


def _householder_wy(hra_u):
    """Return (A, UT) with out = x - (x @ A) @ UT."""
    u = hra_u.astype(np.float32)
    u = u / np.linalg.norm(u, axis=0, keepdims=True)
    U = u.astype(np.float64)
    T = np.zeros((R, R), np.float64)
    for k in range(R):
        T[k, k] = 2.0
        if k:
            T[:k, k] = -2.0 * (T[:k, :k] @ (U[:, :k].T @ U[:, k]))
    A = (U @ T).astype(np.float32)          # [D, R]
    return A, np.ascontiguousarray(u.T)     # [R, D]


J = 2                             # 128-row tiles per block
BLK = J * P                       # 256 rows per block
N_BLKS = ROWS_PER_CORE // BLK     # 4 blocks per core

# drain policy per back unit (j*8+c): D=direct DVE, AG=ACT+GPSIMD,
# AD=ACT+DVE-bf16
_D, _AG, _AD = 0, 1, 2
_POL = [_D, _AG, _D, _AG, _D, _AG, _AD, _D,
        _D, _AG, _D, _AG, _D, _AG, _AD, _D]
POLICY = {i: _POL[i] for i in range(16)}
POLICY_LAST = {i: (_D, _AG, _AD)[i % 3] for i in range(16)}


def _build_program():
    nc = bacc.Bacc(trn_type="TRN2")
    x = nc.dram_tensor("x", (ROWS_PER_CORE, D), BF16, kind="ExternalInput")
    a = nc.dram_tensor("a", (P, D_CHUNKS * R), BF16, kind="ExternalInput")
    ut = nc.dram_tensor("ut", (R, D), BF16, kind="ExternalInput")
    ident = nc.dram_tensor("ident", (P, P), BF16, kind="ExternalInput")
    out = nc.dram_tensor("out", (ROWS_PER_CORE, D), BF16, kind="ExternalOutput")

    xd = x.rearrange("(b j p) d -> b p j d", p=P, j=J)
    od = out.rearrange("(b j p) d -> b p j d", p=P, j=J)

    with tile.TileContext(nc) as tc:
        with (
            tc.tile_pool(name="const", bufs=1) as const,
            tc.tile_pool(name="xp", bufs=4) as x_pool,
            tc.tile_pool(name="xtp", bufs=2) as xt_pool,
            tc.tile_pool(name="tmp", bufs=3) as tmp_pool,
            tc.tile_pool(name="ptp", bufs=2) as pt_pool,
            tc.tile_pool(name="pst", bufs=2, space="PSUM") as pst_pool,
            tc.tile_pool(name="psp", bufs=1, space="PSUM") as psp_pool,
            tc.tile_pool(name="pso", bufs=3, space="PSUM") as pso_pool,
        ):
            # ident first: the warm-up burst depends only on it
            ident_sb = const.tile([P, P], BF16)
            nc.sync.dma_start(ident_sb, ident[:, :])

            # chunks 24-31 of x^T via xbar DMA-transpose on the ACT ring
            # (concurrent with the natural-layout input stream on sync)
            xtd = const.tile([P, D_CHUNKS - DMA_CH0, ROWS_PER_CORE], BF16)
            for k in range(DMA_CH0, D_CHUNKS):
                nc.scalar.dma_start_transpose(
                    xtd[:, k - DMA_CH0, :], x[:, k * P:(k + 1) * P])

            # block-0 leading chunks next, so the first transposes start
            # as soon as they land
            xbs = []
            xb0 = x_pool.tile([P, J, D], BF16, tag="xb")
            xbs.append(xb0)
            h = D // 2
            for j in range(J):
                nc.sync.dma_start(xb0[:, j, :h], xd[0, :, j, :h])

            a_sb = const.tile([P, D_CHUNKS * R], BF16)
            nc.sync.dma_start(a_sb, a[:, :])
            ut_sb = const.tile([R, D], BF16)
            nc.sync.dma_start(ut_sb, ut[:, :])

            for j in range(J):
                nc.sync.dma_start(xb0[:, j, h:], xd[0, :, j, h:])

            # ~4us of ident-only matmuls to open the HAM clock-gate while
            # the input DMAs fill (transpose-mode doesn't count as PE
            # activity, so these must be real matmuls)
            warm_t = pst_pool.tile([P, SG, BLK], BF16, name="ps_t", tag="ps_t")
            nc.tensor.transpose(warm_t[:, 0, :P], ident_sb, ident_sb)
            warm = pso_pool.tile([P, 512], F32, tag="ps_o")
            for _ in range(26):
                nc.tensor.matmul(warm[:, :P], ident_sb, ident_sb,
                                 start=True, stop=True)
            # prime PE's sync-wait on the other two constants
            nc.tensor.matmul(warm[:R, :P], a_sb[:, :R], a_sb[:, :P],
                             start=True, stop=True)
            nc.tensor.matmul(warm[:, :512], ut_sb[:, :P], ut_sb[:, :512],
                             start=True, stop=True)

            # prefetch the remaining block inputs (one 2 MB DMA each)
            for b in range(1, N_BLKS):
                xb = x_pool.tile([P, J, D], BF16, tag="xb")
                xbs.append(xb)
                nc.sync.dma_start(xb, xd[b])

            def back_units(b, pt):
                """yield per-(j,c) update+drain callables"""
                xb = xbs[b]
                last = b == N_BLKS - 1
                policy = POLICY_LAST if last else POLICY

                def unit(j, c):
                    ps_o = pso_pool.tile([P, 512], F32, tag="ps_o")
                    nc.tensor.matmul(
                        ps_o,
                        pt[:, j * P:(j + 1) * P],
                        ut_sb[:, c * 512:(c + 1) * 512],
                        start=True,
                        stop=True,
                    )
                    dst = xb[:, j, c * 512:(c + 1) * 512]
                    pol = policy[j * UPD_CHUNKS + c]
                    if pol == _D:
                        nc.vector.tensor_sub(dst, dst, ps_o)
                    else:
                        t = tmp_pool.tile([P, 512], BF16, tag="tmp")
                        nc.scalar.copy(t, ps_o)
                        if pol == _AG:
                            nc.gpsimd.tensor_sub(dst, dst, t)
                        else:
                            nc.vector.tensor_sub(dst, dst, t)
                    if last:
                        if c % 2 == 1:
                            lo, hi = (c - 1) * 512, (c + 1) * 512
                            nc.gpsimd.dma_start(od[b, :, j, lo:hi],
                                                xb[:, j, lo:hi])
                    elif j == J - 1 and c == UPD_CHUNKS - 1:
                        nc.gpsimd.dma_start(od[b], xb)

                for j in range(J):
                    for c in range(UPD_CHUNKS):
                        yield lambda j=j, c=c: unit(j, c)

            def front_units(b):
                """PE transposes for chunks 0-23 (3 super-groups, one ACT
                f32-pair copy each, proj one group behind); chunks 24-31
                project straight from the DMA-transposed tile."""
                ps_p = psp_pool.tile([R, BLK], F32, tag="ps_p")
                xts = [None] * PE_SGS
                pss = [None] * PE_SGS

                def transposes(sg, half):
                    if half == 0:
                        pss[sg] = pst_pool.tile([P, SG, BLK], BF16,
                                                name="ps_t", tag="ps_t")
                    ps_t = pss[sg]
                    for i in range(SG // 2 * half, SG // 2 * (half + 1)):
                        k = SG * sg + i
                        for j in range(J):
                            nc.tensor.transpose(
                                ps_t[:, i, j * P:(j + 1) * P],
                                xbs[b][:, j, k * P:(k + 1) * P],
                                ident_sb,
                            )
                    if half == 1:
                        xt_g = xt_pool.tile([P, SG, BLK], BF16, tag="xt_g")
                        nc.scalar.copy(xt_g.bitcast(F32), ps_t.bitcast(F32))
                        xts[sg] = xt_g

                def proj(sg, half):
                    for i in range(SG // 2 * half, SG // 2 * (half + 1)):
                        k = SG * sg + i
                        nc.tensor.matmul(
                            ps_p,
                            a_sb[:, k * R:(k + 1) * R],
                            xts[sg][:, i],
                            start=(k == 0),
                            stop=False,
                        )

                def proj_dma(half):
                    for i in range(4 * half, 4 * (half + 1)):
                        k = DMA_CH0 + i
                        nc.tensor.matmul(
                            ps_p,
                            a_sb[:, k * R:(k + 1) * R],
                            xtd[:, i, b * BLK:(b + 1) * BLK],
                            start=False,
                            stop=(k == D_CHUNKS - 1),
                        )

                def finish():
                    pt = pt_pool.tile([R, BLK], BF16, tag="pt")
                    nc.scalar.copy(pt, ps_p)
                    pts[b] = pt

                yield lambda: transposes(0, 0)
                yield lambda: transposes(0, 1)
                for sg in range(1, PE_SGS):
                    yield lambda sg=sg: transposes(sg, 0)
                    yield lambda sg=sg: proj(sg - 1, 0)
                    yield lambda sg=sg: transposes(sg, 1)
                    yield lambda sg=sg: proj(sg - 1, 1)
                yield lambda: proj(PE_SGS - 1, 0)
                yield lambda: proj(PE_SGS - 1, 1)
                yield lambda: proj_dma(0)
                yield lambda: proj_dma(1)
                yield lambda: finish()

            def drain(it):
                for f in it:
                    f()

            pts = {}
            drain(front_units(0))
            for b in range(1, N_BLKS):
                fu = list(front_units(b))       # 15 units
                bu = list(back_units(b - 1, pts[b - 1]))  # 16 units
                order = []
                bi = 0
                for i, f in enumerate(fu):
                    order.append(f)
                    if i >= 1 and bi < len(bu):
                        order.append(bu[bi])
                        bi += 1
                while bi < len(bu):
                    order.append(bu[bi])
                    bi += 1
                drain(order)
            drain(back_units(N_BLKS - 1, pts[N_BLKS - 1]))

    nc.compile()
    return nc


def _get_program():
    if "nc" not in _CACHE:
        _CACHE["nc"] = _build_program()
    return _CACHE["nc"]


def kernel(input, hra_u, **run_kwargs):
    input = np.asarray(input, dtype=np.float32)
    hra_u = np.asarray(hra_u, dtype=np.float32)

    A, UT = _householder_wy(hra_u)
    # pack A [D, R] so partition p holds A[c*128+p, :] at free offset c*R
    a_packed = np.ascontiguousarray(
        A.reshape(D_CHUNKS, P, R).transpose(1, 0, 2).reshape(P, D_CHUNKS * R)
    ).astype(NP_BF16)
    ut_b = UT.astype(NP_BF16)
    ident = np.eye(P, dtype=np.float32).astype(NP_BF16)

    x_flat = np.ascontiguousarray(input.reshape(ROWS, D)).astype(NP_BF16)
    in_maps = [
        {
            "x": x_flat[c * ROWS_PER_CORE:(c + 1) * ROWS_PER_CORE],
            "a": a_packed,
            "ut": ut_b,
            "ident": ident,
        }
        for c in range(N_CORES)
    ]

    nc = _get_program()
    res = run_bass_kernel_spmd(nc, in_maps, core_ids=list(range(N_CORES)),
                               **run_kwargs)
    out = np.concatenate([r["out"] for r in res.results], axis=0)
    if run_kwargs:
        kernel.last_results = res
    return out.astype(np.float32).reshape(B, S, D)


# revision 11
# speedup vs baseline: 1.3841x; 1.3841x over previous
"""HRA (Householder Reflection Adaptation) forward kernel for Trainium2.

Math: out = x @ Q with Q = prod_i (I - 2 u_i u_i^T), u_i = normalized columns
of hra_u [4096, 8].  Using the compact WY representation:
    Q = I - U T U^T      (T upper-triangular 8x8, diag=2)
    out = x - (x @ A) @ U^T,   A = U @ T
so the device only does two skinny matmuls per tile plus a subtract.

Sharding: data-parallel over rows. x [4,2048,4096] -> [8192, 4096]; each of
8 cores gets 1024 contiguous rows. A and U^T are tiny and replicated.

Everything runs in bf16 (inputs quantized host-side; |err| ~ 5e-3 rel, well
inside the 2e-2 gate): HBM traffic halves (16.8 MB/core, ~47 us roofline)
and PE transposes run at 1 cycle/row (vs 2 for f32).

Per-core structure (4 blocks x 256 rows, software-pipelined):
  front(b): per 8-chunk super-group: 16 PE transposes -> 2-bank bf16 PSUM
    strip, one ACT copy (read as f32 pairs to halve the element count)
    -> SBUF x^T; the proj matmuls P^T[8,256] += A_k^T xT_k run one
    super-group behind so the copy latency is hidden.
  back(b-1), interleaved 2 units per front yield: per-(j,c) [128,512]
    update matmul, then the PSUM drain by per-unit policy:
      D  : direct DVE subtract (PSUM f32, 1x)
      AG : ACT copy -> bf16, subtract on the otherwise-idle GPSIMD
      AD : ACT copy -> bf16, cheap 2x all-bf16 DVE subtract
    so the drain spreads across three engines.
  warm-up: ~26 identity matmuls (gated only on the tiny first DMA) open
    the HAM clock-gate during the input fill.
  output DMAs issue from GPSIMD (SWDGE, free descriptor generation); the
  last block streams out in quarter pieces to shrink the tail.
"""

import os
import sys

for _p in ("/opt/trn_rl_repo", "/root/.axon_site", "/root/.axon_site/_ro/trn_rl_repo",
           "/root/.axon_site/_ro/pypackages"):
    if os.path.isdir(_p) and _p not in sys.path:
        sys.path.append(_p)

import numpy as np
import ml_dtypes

import concourse.bass as bass
import concourse.mybir as mybir
import concourse.tile as tile
from concourse import bacc
from concourse.bass_utils import run_bass_kernel_spmd

B, S, D, R = 4, 2048, 4096, 8
N_CORES = 8
ROWS = B * S                      # 8192
ROWS_PER_CORE = ROWS // N_CORES   # 1024
P = 128
D_CHUNKS = D // P                 # 32
SG = 8                            # d-chunks per transpose super-group
N_SG = D_CHUNKS // SG             # 4 super-groups per block
UPD_CHUNKS = D // 512             # 8 update units per j

F32 = mybir.dt.float32
BF16 = mybir.dt.bfloat16
NP_BF16 = ml_dtypes.bfloat16

_CACHE = {}


def _householder_wy(hra_u):
    """Return (A, UT) with out = x - (x @ A) @ UT."""
    u = hra_u.astype(np.float32)
    u = u / np.linalg.norm(u, axis=0, keepdims=True)
    U = u.astype(np.float64)
    T = np.zeros((R, R), np.float64)
    for k in range(R):
        T[k, k] = 2.0
        if k:
            T[:k, k] = -2.0 * (T[:k, :k] @ (U[:, :k].T @ U[:, k]))
    A = (U @ T).astype(np.float32)          # [D, R]
    return A, np.ascontiguousarray(u.T)     # [R, D]


J = 2                             # 128-row tiles per block
BLK = J * P                       # 256 rows per block
N_BLKS = ROWS_PER_CORE // BLK     # 4 blocks per core

# drain policy per back unit (j*8+c)
_D, _AG, _AD = 0, 1, 2
_POL = [_D, _D, _AG, _D, _D, _D, _AD, _D,
        _D, _D, _AG, _D, _D, _D, _AD, _D]
POLICY = {i: _POL[i] for i in range(16)}
POLICY_LAST = {i: (_D, _AG, _D, _AD)[i % 4] for i in range(16)}


def _build_program():
    nc = bacc.Bacc(trn_type="TRN2")
    x = nc.dram_tensor("x", (ROWS_PER_CORE, D), BF16, kind="ExternalInput")
    a = nc.dram_tensor("a", (P, D_CHUNKS * R), BF16, kind="ExternalInput")
    ut = nc.dram_tensor("ut", (R, D), BF16, kind="ExternalInput")
    ident = nc.dram_tensor("ident", (P, P), BF16, kind="ExternalInput")
    out = nc.dram_tensor("out", (ROWS_PER_CORE, D), BF16, kind="ExternalOutput")

    xd = x.rearrange("(b j p) d -> b p j d", p=P, j=J)
    od = out.rearrange("(b j p) d -> b p j d", p=P, j=J)

    with tile.TileContext(nc) as tc:
        with (
            tc.tile_pool(name="const", bufs=1) as const,
            tc.tile_pool(name="xp", bufs=4) as x_pool,
            tc.tile_pool(name="xtp", bufs=2) as xt_pool,
            tc.tile_pool(name="tmp", bufs=3) as tmp_pool,
            tc.tile_pool(name="ptp", bufs=2) as pt_pool,
            tc.tile_pool(name="pst", bufs=2, space="PSUM") as pst_pool,
            tc.tile_pool(name="psp", bufs=1, space="PSUM") as psp_pool,
            tc.tile_pool(name="pso", bufs=3, space="PSUM") as pso_pool,
        ):
            # ident first: the warm-up burst depends only on it
            ident_sb = const.tile([P, P], BF16)
            nc.sync.dma_start(ident_sb, ident[:, :])

            # block-0 leading halves next, so the first transposes start
            # as soon as they land
            xbs = []
            xb0 = x_pool.tile([P, J, D], BF16, tag="xb")
            xbs.append(xb0)
            h = D // 2
            for j in range(J):
                nc.sync.dma_start(xb0[:, j, :h], xd[0, :, j, :h])

            a_sb = const.tile([P, D_CHUNKS * R], BF16)
            nc.sync.dma_start(a_sb, a[:, :])
            ut_sb = const.tile([R, D], BF16)
            nc.sync.dma_start(ut_sb, ut[:, :])

            for j in range(J):
                nc.sync.dma_start(xb0[:, j, h:], xd[0, :, j, h:])

            # ~4us of ident-only matmuls to open the HAM clock-gate while
            # the input DMAs fill (transpose-mode doesn't count as PE
            # activity, so these must be real matmuls)
            warm_t = pst_pool.tile([P, SG, BLK], BF16, name="ps_t", tag="ps_t")
            nc.tensor.transpose(warm_t[:, 0, :P], ident_sb, ident_sb)
            warm = pso_pool.tile([P, 512], F32, tag="ps_o")
            for _ in range(26):
                nc.tensor.matmul(warm[:, :P], ident_sb, ident_sb,
                                 start=True, stop=True)
            # prime PE's sync-wait on the other two constants
            nc.tensor.matmul(warm[:R, :P], a_sb[:, :R], a_sb[:, :P],
                             start=True, stop=True)
            nc.tensor.matmul(warm[:, :512], ut_sb[:, :P], ut_sb[:, :512],
                             start=True, stop=True)

            # prefetch the remaining block inputs (one 2 MB DMA each)
            for b in range(1, N_BLKS):
                xb = x_pool.tile([P, J, D], BF16, tag="xb")
                xbs.append(xb)
                nc.sync.dma_start(xb, xd[b])

            def back_units(b, pt):
                """yield per-(j,c) update+drain callables"""
                xb = xbs[b]
                last = b == N_BLKS - 1
                policy = POLICY_LAST if last else POLICY

                def unit(j, c):
                    ps_o = pso_pool.tile([P, 512], F32, tag="ps_o")
                    nc.tensor.matmul(
                        ps_o,
                        pt[:, j * P:(j + 1) * P],
                        ut_sb[:, c * 512:(c + 1) * 512],
                        start=True,
                        stop=True,
                    )
                    dst = xb[:, j, c * 512:(c + 1) * 512]
                    pol = policy[j * UPD_CHUNKS + c]
                    if pol == _D:
                        nc.vector.tensor_sub(dst, dst, ps_o)
                    else:
                        t = tmp_pool.tile([P, 512], BF16, tag="tmp")
                        nc.scalar.copy(t, ps_o)
                        if pol == _AG:
                            nc.gpsimd.tensor_sub(dst, dst, t)
                        else:
                            nc.vector.tensor_sub(dst, dst, t)
                    if last:
                        if c % 2 == 1:
                            lo, hi = (c - 1) * 512, (c + 1) * 512
                            nc.gpsimd.dma_start(od[b, :, j, lo:hi],
                                                xb[:, j, lo:hi])
                    elif j == J - 1 and c == UPD_CHUNKS - 1:
                        nc.gpsimd.dma_start(od[b], xb)

                for j in range(J):
                    for c in range(UPD_CHUNKS):
                        yield lambda j=j, c=c: unit(j, c)

            def front_units(b):
                """Super-group pipeline: 16 transposes land a [128,8,256]
                bf16 PSUM strip, one ACT copy (read as f32 pairs) moves it
                to SBUF, and the 8 proj matmuls of the PREVIOUS super-group
                run under that copy's latency.  pt lands in pts[b]."""
                ps_p = psp_pool.tile([R, BLK], F32, tag="ps_p")
                xts = [None] * N_SG

                def transposes(sg):
                    ps_t = pst_pool.tile([P, SG, BLK], BF16,
                                         name="ps_t", tag="ps_t")
                    for i in range(SG):
                        k = SG * sg + i
                        for j in range(J):
                            nc.tensor.transpose(
                                ps_t[:, i, j * P:(j + 1) * P],
                                xbs[b][:, j, k * P:(k + 1) * P],
                                ident_sb,
                            )
                    xt_g = xt_pool.tile([P, SG, BLK], BF16, tag="xt_g")
                    nc.scalar.copy(xt_g.bitcast(F32), ps_t.bitcast(F32))
                    xts[sg] = xt_g

                def proj(sg):
                    for i in range(SG):
                        k = SG * sg + i
                        nc.tensor.matmul(
                            ps_p,
                            a_sb[:, k * R:(k + 1) * R],
                            xts[sg][:, i],
                            start=(k == 0),
                            stop=(k == D_CHUNKS - 1),
                        )

                def finish():
                    pt = pt_pool.tile([R, BLK], BF16, tag="pt")
                    nc.scalar.copy(pt, ps_p)
                    pts[b] = pt

                yield lambda: transposes(0)
                for sg in range(1, N_SG):
                    yield lambda sg=sg: transposes(sg)
                    yield lambda sg=sg: proj(sg - 1)
                yield lambda: proj(N_SG - 1)
                yield lambda: finish()

            def drain(it):
                for f in it:
                    f()

            pts = {}
            drain(front_units(0))
            for b in range(1, N_BLKS):
                fu = list(front_units(b))       # 9 units
                bu = list(back_units(b - 1, pts[b - 1]))  # 16 units
                # interleave: 2 back units after each front unit until spent
                order = []
                bi = 0
                for f in fu:
                    order.append(f)
                    for _ in range(2):
                        if bi < len(bu):
                            order.append(bu[bi])
                            bi += 1
                while bi < len(bu):
                    order.append(bu[bi])
                    bi += 1
                drain(order)
            drain(back_units(N_BLKS - 1, pts[N_BLKS - 1]))

    nc.compile()
    return nc


def _get_program():
    if "nc" not in _CACHE:
        _CACHE["nc"] = _build_program()
    return _CACHE["nc"]


def kernel(input, hra_u, **run_kwargs):
    input = np.asarray(input, dtype=np.float32)
    hra_u = np.asarray(hra_u, dtype=np.float32)

    A, UT = _householder_wy(hra_u)
    # pack A [D, R] so partition p holds A[c*128+p, :] at free offset c*R
    a_packed = np.ascontiguousarray(
        A.reshape(D_CHUNKS, P, R).transpose(1, 0, 2).reshape(P, D_CHUNKS * R)
    ).astype(NP_BF16)
    ut_b = UT.astype(NP_BF16)
    ident = np.eye(P, dtype=np.float32).astype(NP_BF16)

    x_flat = np.ascontiguousarray(input.reshape(ROWS, D)).astype(NP_BF16)
    in_maps = [
        {
            "x": x_flat[c * ROWS_PER_CORE:(c + 1) * ROWS_PER_CORE],
            "a": a_packed,
            "ut": ut_b,
            "ident": ident,
        }
        for c in range(N_CORES)
    ]

    nc = _get_program()
    res = run_bass_kernel_spmd(nc, in_maps, core_ids=list(range(N_CORES)),
                               **run_kwargs)
    out = np.concatenate([r["out"] for r in res.results], axis=0)
    if run_kwargs:
        kernel.last_results = res
    return out.astype(np.float32).reshape(B, S, D)


# revision 12
# speedup vs baseline: 1.4303x; 1.0334x over previous
"""HRA (Householder Reflection Adaptation) forward kernel for Trainium2.

Math: out = x @ Q with Q = prod_i (I - 2 u_i u_i^T), u_i = normalized columns
of hra_u [4096, 8].  Using the compact WY representation:
    Q = I - U T U^T      (T upper-triangular 8x8, diag=2)
    out = x - (x @ A) @ U^T,   A = U @ T
so the device only does two skinny matmuls per tile plus a subtract.

Sharding: data-parallel over rows. x [4,2048,4096] -> [8192, 4096]; each of
8 cores gets 1024 contiguous rows. A and U^T are tiny and replicated.

Everything runs in bf16 (inputs quantized host-side; |err| ~ 5e-3 rel, well
inside the 2e-2 gate): HBM traffic halves (16.8 MB/core, ~47 us roofline)
and PE transposes run at 1 cycle/row (vs 2 for f32).

Per-core structure (4 blocks x 256 rows, software-pipelined):
  front(b): per 8-chunk super-group: 16 PE transposes -> 2-bank bf16 PSUM
    strip, one ACT copy (read as f32 pairs to halve the element count)
    -> SBUF x^T; the proj matmuls P^T[8,256] += A_k^T xT_k run one
    super-group behind so the copy latency is hidden.
  back(b-1), interleaved 2 units per front yield: per-(j,c) [128,512]
    update matmul; the PSUM drain is a direct DVE subtract for most units
    and ACT-copy + cheap all-bf16 DVE subtract for a few, so the drain
    spreads across both engines; the last block alternates so the tail
    drains in parallel.
  warm-up: ~4us of matmuls during the initial DMA fill opens the PE HAM
    clock-gate before the first real block.
  output DMAs issue from GPSIMD (SWDGE, free descriptor generation); the
  last block streams out in quarter pieces to shrink the tail.
"""

import os
import sys

for _p in ("/opt/trn_rl_repo", "/root/.axon_site", "/root/.axon_site/_ro/trn_rl_repo",
           "/root/.axon_site/_ro/pypackages"):
    if os.path.isdir(_p) and _p not in sys.path:
        sys.path.append(_p)

import numpy as np
import ml_dtypes

import concourse.bass as bass
import concourse.mybir as mybir
import concourse.tile as tile
from concourse import bacc
from concourse.bass_utils import run_bass_kernel_spmd

B, S, D, R = 4, 2048, 4096, 8
N_CORES = 8
ROWS = B * S                      # 8192
ROWS_PER_CORE = ROWS // N_CORES   # 1024
P = 128
D_CHUNKS = D // P                 # 32
SG = 8                            # d-chunks per transpose super-group
N_SG = D_CHUNKS // SG             # 4 super-groups per block
UPD_CHUNKS = D // 512             # 8 update units per j

F32 = mybir.dt.float32
BF16 = mybir.dt.bfloat16
NP_BF16 = ml_dtypes.bfloat16

_CACHE = {}


def _householder_wy(hra_u):
    """Return (A, UT) with out = x - (x @ A) @ UT."""
    u = hra_u.astype(np.float32)
    u = u / np.linalg.norm(u, axis=0, keepdims=True)
    U = u.astype(np.float64)
    T = np.zeros((R, R), np.float64)
    for k in range(R):
        T[k, k] = 2.0
        if k:
            T[:k, k] = -2.0 * (T[:k, :k] @ (U[:, :k].T @ U[:, k]))
    A = (U @ T).astype(np.float32)          # [D, R]
    return A, np.ascontiguousarray(u.T)     # [R, D]


J = 2                             # 128-row tiles per block
BLK = J * P                       # 256 rows per block
N_BLKS = ROWS_PER_CORE // BLK     # 4 blocks per core

# back-units (j*8+c) whose PSUM drain goes ACT-copy + bf16 DVE sub
# instead of a direct DVE PSUM subtract; the last block alternates so the
# tail drains on both engines in parallel
OFFLOAD = {2, 6, 10, 14}
OFFLOAD_LAST = {1, 3, 5, 7, 9, 11, 13, 15}


def _build_program():
    nc = bacc.Bacc(trn_type="TRN2")
    x = nc.dram_tensor("x", (ROWS_PER_CORE, D), BF16, kind="ExternalInput")
    a = nc.dram_tensor("a", (P, D_CHUNKS * R), BF16, kind="ExternalInput")
    ut = nc.dram_tensor("ut", (R, D), BF16, kind="ExternalInput")
    ident = nc.dram_tensor("ident", (P, P), BF16, kind="ExternalInput")
    out = nc.dram_tensor("out", (ROWS_PER_CORE, D), BF16, kind="ExternalOutput")

    xd = x.rearrange("(b j p) d -> b p j d", p=P, j=J)
    od = out.rearrange("(b j p) d -> b p j d", p=P, j=J)

    with tile.TileContext(nc) as tc:
        with (
            tc.tile_pool(name="const", bufs=1) as const,
            tc.tile_pool(name="xp", bufs=4) as x_pool,
            tc.tile_pool(name="xtp", bufs=2) as xt_pool,
            tc.tile_pool(name="tmp", bufs=2) as tmp_pool,
            tc.tile_pool(name="ptp", bufs=2) as pt_pool,
            tc.tile_pool(name="pst", bufs=2, space="PSUM") as pst_pool,
            tc.tile_pool(name="psp", bufs=1, space="PSUM") as psp_pool,
            tc.tile_pool(name="pso", bufs=3, space="PSUM") as pso_pool,
        ):
            # ident first: the warm-up burst depends only on it
            ident_sb = const.tile([P, P], BF16)
            nc.sync.dma_start(ident_sb, ident[:, :])

            # block-0 leading halves next, so the first transposes start
            # as soon as the leading chunks land
            xbs = []
            xb0 = x_pool.tile([P, J, D], BF16, tag="xb")
            xbs.append(xb0)
            h = D // 2
            for j in range(J):
                nc.sync.dma_start(xb0[:, j, :h], xd[0, :, j, :h])

            a_sb = const.tile([P, D_CHUNKS * R], BF16)
            nc.sync.dma_start(a_sb, a[:, :])
            ut_sb = const.tile([R, D], BF16)
            nc.sync.dma_start(ut_sb, ut[:, :])

            for j in range(J):
                nc.sync.dma_start(xb0[:, j, h:], xd[0, :, j, h:])

            # Prime PE on each constant (one sync-wait per LDWEIGHTS), then
            # ~4us of matmuls during the DMA fill to open the HAM clock-gate.
            warm_t = pst_pool.tile([P, SG, BLK], BF16, name="ps_t", tag="ps_t")
            nc.tensor.transpose(warm_t[:, 0, :P], ident_sb, ident_sb)
            warm = pso_pool.tile([P, 512], F32, tag="ps_o")
            nc.tensor.matmul(warm[:R, :P], a_sb[:, :R], a_sb[:, :P],
                             start=True, stop=True)
            for _ in range(10):
                nc.tensor.matmul(warm[:, :512], ut_sb[:, :P], ut_sb[:, :512],
                                 start=True, stop=True)

            # prefetch the remaining block inputs (one 2 MB DMA each)
            for b in range(1, N_BLKS):
                xb = x_pool.tile([P, J, D], BF16, tag="xb")
                xbs.append(xb)
                nc.sync.dma_start(xb, xd[b])

            def back_units(b, pt):
                """yield per-(j,c) update+subtract callables; the final
                block streams its output in quarter pieces to cut the tail"""
                xb = xbs[b]
                last = b == N_BLKS - 1
                off = OFFLOAD_LAST if last else OFFLOAD

                def unit(j, c):
                    ps_o = pso_pool.tile([P, 512], F32, tag="ps_o")
                    nc.tensor.matmul(
                        ps_o,
                        pt[:, j * P:(j + 1) * P],
                        ut_sb[:, c * 512:(c + 1) * 512],
                        start=True,
                        stop=True,
                    )
                    dst = xb[:, j, c * 512:(c + 1) * 512]
                    if (j * UPD_CHUNKS + c) in off:
                        t = tmp_pool.tile([P, 512], BF16, tag="tmp")
                        nc.scalar.copy(t, ps_o)
                        nc.vector.tensor_sub(dst, dst, t)
                    else:
                        nc.vector.tensor_sub(dst, dst, ps_o)
                    if last:
                        if (c + 1) % 2 == 0:
                            lo, hi = (c - 1) * 512, (c + 1) * 512
                            nc.gpsimd.dma_start(od[b, :, j, lo:hi],
                                                xb[:, j, lo:hi])
                    elif c == UPD_CHUNKS - 1 and j == J - 1:
                        nc.gpsimd.dma_start(od[b], xb)

                for j in range(J):
                    for c in range(UPD_CHUNKS):
                        yield lambda j=j, c=c: unit(j, c)

            def front_units(b):
                """Super-group pipeline: 16 transposes land a [128,8,256]
                bf16 PSUM strip, one ACT copy (read as f32 pairs) moves it
                to SBUF, and the 8 proj matmuls of the PREVIOUS super-group
                run under that copy's latency.  pt lands in pts[b]."""
                ps_p = psp_pool.tile([R, BLK], F32, tag="ps_p")
                xts = [None] * N_SG

                def transposes(sg):
                    ps_t = pst_pool.tile([P, SG, BLK], BF16,
                                         name="ps_t", tag="ps_t")
                    for i in range(SG):
                        k = SG * sg + i
                        for j in range(J):
                            nc.tensor.transpose(
                                ps_t[:, i, j * P:(j + 1) * P],
                                xbs[b][:, j, k * P:(k + 1) * P],
                                ident_sb,
                            )
                    xt_g = xt_pool.tile([P, SG, BLK], BF16, tag="xt_g")
                    nc.scalar.copy(xt_g.bitcast(F32), ps_t.bitcast(F32))
                    xts[sg] = xt_g

                def proj(sg):
                    for i in range(SG):
                        k = SG * sg + i
                        nc.tensor.matmul(
                            ps_p,
                            a_sb[:, k * R:(k + 1) * R],
                            xts[sg][:, i],
                            start=(k == 0),
                            stop=(k == D_CHUNKS - 1),
                        )

                def finish():
                    pt = pt_pool.tile([R, BLK], BF16, tag="pt")
                    nc.vector.tensor_copy(pt, ps_p)
                    pts[b] = pt

                yield lambda: transposes(0)
                for sg in range(1, N_SG):
                    yield lambda sg=sg: transposes(sg)
                    yield lambda sg=sg: proj(sg - 1)
                yield lambda: proj(N_SG - 1)
                yield lambda: finish()

            def drain(it):
                for f in it:
                    f()

            pts = {}
            drain(front_units(0))
            for b in range(1, N_BLKS):
                fu = list(front_units(b))       # 9 units
                bu = list(back_units(b - 1, pts[b - 1]))  # 16 units
                # interleave: 2 back units after each front unit until spent
                order = []
                bi = 0
                for f in fu:
                    order.append(f)
                    for _ in range(2):
                        if bi < len(bu):
                            order.append(bu[bi])
                            bi += 1
                while bi < len(bu):
                    order.append(bu[bi])
                    bi += 1
                drain(order)
            drain(back_units(N_BLKS - 1, pts[N_BLKS - 1]))

    nc.compile()
    return nc


def _get_program():
    if "nc" not in _CACHE:
        _CACHE["nc"] = _build_program()
    return _CACHE["nc"]


def kernel(input, hra_u, **run_kwargs):
    input = np.asarray(input, dtype=np.float32)
    hra_u = np.asarray(hra_u, dtype=np.float32)

    A, UT = _householder_wy(hra_u)
    # pack A [D, R] so partition p holds A[c*128+p, :] at free offset c*R
    a_packed = np.ascontiguousarray(
        A.reshape(D_CHUNKS, P, R).transpose(1, 0, 2).reshape(P, D_CHUNKS * R)
    ).astype(NP_BF16)
    ut_b = UT.astype(NP_BF16)
    ident = np.eye(P, dtype=np.float32).astype(NP_BF16)

    x_flat = np.ascontiguousarray(input.reshape(ROWS, D)).astype(NP_BF16)
    in_maps = [
        {
            "x": x_flat[c * ROWS_PER_CORE:(c + 1) * ROWS_PER_CORE],
            "a": a_packed,
            "ut": ut_b,
            "ident": ident,
        }
        for c in range(N_CORES)
    ]

    nc = _get_program()
    res = run_bass_kernel_spmd(nc, in_maps, core_ids=list(range(N_CORES)),
                               **run_kwargs)
    out = np.concatenate([r["out"] for r in res.results], axis=0)
    if run_kwargs:
        kernel.last_results = res
    return out.astype(np.float32).reshape(B, S, D)
